# revision 5
# baseline (speedup 1.0000x reference)
"""GCN node classification on 8 Trainium2 NeuronCores (Bass/Tile).

Strategy (dst-sharded graph parallel), v2:
  - Nodes padded to 100352 = 8 * 12544; core c owns dst nodes
    [c*12544, (c+1)*12544)  (98 tiles of 128).
  - Per layer: each core computes xw = g_own @ W on PE, AllGather makes the
    full [100352, F] feature table resident on every core's HBM.
  - Edges are bucketed by (dst-tile, 25088-row src window) and sorted by
    src. Per bucket ("run"): one dma_gather pulls the source rows (int16
    idx relative to the window); a host-precomputed selection matrix
    stream M[e,d] = coef[e] * (d == dst_local[e]) is DMA'd from HBM, and
    PE accumulates psum += M^T @ Y over the run's 128-edge chunks —
    the weighted segment sum. Runs flush psum into an SBUF aggregate.
    Run lengths are the max over the 8 cores (SPMD uniform schedule);
    shorter cores pad with idx 0 and zero M rows. The descriptor-
    generation rate (~7 ns/idx on GPSIMD) is the kernel's bottleneck, so
    no 128-alignment padding and self-loops stay out of the gather.
  - Self-loop term: per tile, ACT rescales the core's own xw rows
    (scale = 2*dinv^2 per node) read back from the collective input.
  - Epilogue per tile: + self + bias, + residual, erf-GELU, PE transpose,
    next layer's matmul, DMA into the next collective's input buffer.
"""
import sys

sys.path.insert(0, "/opt/trn_rl_repo")

import numpy as np

import concourse.bass as bass  # noqa: E402
import concourse.tile as tile  # noqa: E402
from concourse import bacc, mybir  # noqa: E402
from concourse.bass_utils import run_bass_kernel_spmd  # noqa: E402

NCORES = 8
F = 128          # feature width (all layers padded to 128)
TILES = 98       # dst tiles per core
OWN = TILES * 128            # 12544 nodes per core
NT = NCORES * OWN            # 100352 padded nodes
NWIN = 4
WIN = 25088                  # src window (int16-addressable, < 32768)
C_OUT = 40
YBUFS = 4
MBUFS = 4
NQUSE = 4


# --------------------------------------------------------------------------
# host-side preprocessing
# --------------------------------------------------------------------------

def preprocess(x, edge_index, n_real):
    """Shard + schedule.

    Returns (per_core, runs, CHX, IDXC) where
      runs: list of (q, t, R, nch, idx_off16, m_off) shared by all cores;
            R = padded run length (multiple of 16), nch = ceil(R/128).
      per_core: dict with idx16 [128, IDXC], M [128, CHX*128] f32,
                selfw [128, TILES].
    """
    src = np.asarray(edge_index[0], dtype=np.int64)
    dst = np.asarray(edge_index[1], dtype=np.int64)

    deg = np.bincount(dst, minlength=NT).astype(np.float32) + 2.0
    dinv = 1.0 / np.sqrt(deg)
    coef_a = (dinv[src] * dinv[dst]).astype(np.float32)

    core = dst // OWN
    dstl_a = dst - core * OWN
    t_a = dstl_a >> 7
    dloc_a = (dstl_a & 127).astype(np.int64)
    q_a = np.minimum(src // WIN, NWIN - 1)
    idxrel_a = (src - q_a * WIN).astype(np.int64)
    assert idxrel_a.max() < 32768

    counts = np.zeros((NCORES, TILES, NWIN), dtype=np.int64)
    np.add.at(counts, (core, t_a, q_a), 1)
    Lmax = counts.max(axis=0)                       # [TILES, NWIN]
    R_tq = ((Lmax + 15) // 16 * 16).astype(np.int64)

    # run list in q-major order
    runs = []
    idx_off16 = 0
    m_off = 0
    for q in range(NWIN):
        for t in range(TILES):
            R = int(R_tq[t, q])
            if R == 0:
                continue
            nch = (R + 127) // 128
            runs.append((q, t, R, nch, idx_off16, m_off))
            idx_off16 += R // 16
            m_off += nch
    CHX = m_off
    IDXC = idx_off16

    order = np.lexsort((idxrel_a, t_a, q_a, core))
    src_s = idxrel_a[order]
    core_s = core[order]
    t_s = t_a[order]
    q_s = q_a[order]
    dloc_s = dloc_a[order]
    coef_s = coef_a[order]

    run_pos = {(q, t): i for i, (q, t, *_r) in enumerate(runs)}

    per_core = []
    for c in range(NCORES):
        sel = core_s == c
        ci, ct, cq = src_s[sel], t_s[sel], q_s[sel]
        cd, cc = dloc_s[sel], coef_s[sel]
        idx16 = np.zeros(IDXC * 16, dtype=np.int16)
        M = np.zeros((CHX * 128, 128), dtype=np.float32)
        key = cq * TILES + ct
        bounds = np.flatnonzero(np.r_[True, key[1:] != key[:-1], True])
        for bi in range(len(bounds) - 1):
            lo, hi = bounds[bi], bounds[bi + 1]
            q0, t0 = int(cq[lo]), int(ct[lo])
            _, _, R, nch, io16, mo = runs[run_pos[(q0, t0)]]
            n = hi - lo
            assert n <= R
            idx16[io16 * 16: io16 * 16 + n] = ci[lo:hi]
            rows = mo * 128 + np.arange(n)
            M[rows, cd[lo:hi]] = cc[lo:hi]
        # wrap idx per run: idx j of run -> [j%16, j//16], replicate x8
        idx_w = np.zeros((128, IDXC), dtype=np.int16)
        for (q0, t0, R, nch, io16, mo) in runs:
            blk = idx16[io16 * 16: io16 * 16 + R].reshape(-1, 16).T
            idx_w[:, io16: io16 + R // 16] = np.tile(blk, (8, 1))
        # M layout: [128 edge-part, CHX*128]: edge e of chunk j at
        # [e, j*128 + d]
        M_w = M.reshape(CHX, 128, 128).transpose(1, 0, 2).reshape(128, CHX * 128)
        own = slice(c * OWN, (c + 1) * OWN)
        selfw = (2.0 * dinv[own] * dinv[own]).astype(np.float32)
        per_core.append({
            "idx16": np.ascontiguousarray(idx_w),
            "M": np.ascontiguousarray(M_w),
            "selfw": selfw.reshape(TILES, 128).T.copy(),
        })

    return per_core, runs, CHX, IDXC


# --------------------------------------------------------------------------
# bass program
# --------------------------------------------------------------------------

def build(runs, CHX, IDXC):
    nc = bacc.Bacc("TRN2", target_bir_lowering=False, debug=False,
                   num_devices=NCORES, num_swdge_queues=NQUSE)

    xT_in = nc.dram_tensor("xT", [128, OWN], mybir.dt.float32, kind="ExternalInput")
    idx16_in = nc.dram_tensor("idx16", [128, IDXC], mybir.dt.int16, kind="ExternalInput")
    m_in = nc.dram_tensor("M", [128, CHX * 128], mybir.dt.float32, kind="ExternalInput")
    selfw_in = nc.dram_tensor("selfw", [128, TILES], mybir.dt.float32, kind="ExternalInput")
    w_in = [nc.dram_tensor(f"W{l}", [128, 128], mybir.dt.float32, kind="ExternalInput")
            for l in range(4)]
    b_in = [nc.dram_tensor(f"b{l}", [128, 128], mybir.dt.float32, kind="ExternalInput")
            for l in range(4)]
    ident_in = nc.dram_tensor("ident", [128, 128], mybir.dt.float32, kind="ExternalInput")
    out_dram = nc.dram_tensor("out", [OWN, 128], mybir.dt.float32, kind="ExternalOutput")

    first_q, last_q = {}, {}
    for (q, t, *_r) in runs:
        first_q.setdefault(t, q)
        last_q[t] = q
    max_nch = max(r[3] for r in runs)

    with tile.TileContext(nc) as tc:
        with (
            tc.tile_pool(name="persist", bufs=1) as pers,
            tc.tile_pool(name="ybuf", bufs=YBUFS) as yp,
            tc.tile_pool(name="mbuf", bufs=MBUFS) as mp,
            tc.tile_pool(name="runp", bufs=4, space="PSUM") as rp,
            tc.tile_pool(name="epip", bufs=2, space="PSUM") as ep,
            tc.tile_pool(name="etmp", bufs=4) as et,
            tc.tile_pool(name="xtile", bufs=4) as xp,
            tc.tile_pool(name="dram", bufs=1, space="DRAM") as dp,
        ):
            # ---- persistent SBUF ----
            idx_t = pers.tile([128, IDXC], mybir.dt.int16, tag="idx")
            nc.sync.dma_start(idx_t[:], idx16_in[:])
            selfw_t = pers.tile([128, TILES], mybir.dt.float32, tag="selfw")
            nc.sync.dma_start(selfw_t[:], selfw_in[:])
            ident_t = pers.tile([128, 128], mybir.dt.float32, tag="ident")
            nc.sync.dma_start(ident_t[:], ident_in[:])
            w_t, b_t = [], []
            for l in range(4):
                wt = pers.tile([128, 128], mybir.dt.float32, tag=f"w{l}")
                nc.sync.dma_start(wt[:], w_in[l][:])
                w_t.append(wt)
                bt = pers.tile([128, 128], mybir.dt.float32, tag=f"b{l}")
                nc.sync.dma_start(bt[:], b_in[l][:])
                b_t.append(bt)
            agg_t = pers.tile([128, TILES * 128], mybir.dt.float32, tag="agg")
            g_t = pers.tile([128, TILES * 128], mybir.dt.float32, tag="g")

            # zero the gather slots once (short-count gathers leave stale
            # tails; M zero rows null them, but stale uninit SBUF could be
            # NaN and 0*NaN = NaN)
            for _ in range(YBUFS):
                yz = yp.tile([128, max_nch, 128], mybir.dt.float32, tag="y")
                nc.vector.memset(yz[:], 0.0)

            # ---- collective buffers ----
            cc_in = [dp.tile([OWN, 128], mybir.dt.float32, tag=f"ccin{l}",
                             name=f"ccin{l}") for l in range(4)]
            cc_out = [dp.tile([NT, 128], mybir.dt.float32, tag=f"ccout{l}",
                              name=f"ccout{l}", addr_space="Shared")
                      for l in range(4)]

            def make_xw(l, lhsT_tile, t):
                pxw = ep.tile([128, 128], mybir.dt.float32, space="PSUM", tag="pxw")
                nc.tensor.matmul(out=pxw[:], lhsT=lhsT_tile[:], rhs=w_t[l][:],
                                 start=True, stop=True)
                xw_sb = et.tile([128, 128], mybir.dt.float32, tag="xwsb")
                nc.scalar.activation(xw_sb[:], pxw[:],
                                     mybir.ActivationFunctionType.Copy)
                nc.sync.dma_start(cc_in[l][t * 128:(t + 1) * 128, :], xw_sb[:])

            # ---- layer 0 pre-phase: xw0 = x @ W0 ----
            for t in range(TILES):
                xt = xp.tile([128, 128], mybir.dt.float32, tag="xt")
                nc.sync.dma_start(xt[:], xT_in[:, t * 128:(t + 1) * 128])
                make_xw(0, xt, t)

            gq = [0]

            def epilogue(l, t):
                agg_sl = agg_t[:, t * 128:(t + 1) * 128]
                g_sl = g_t[:, t * 128:(t + 1) * 128]
                # self-loop: scale own xw rows by 2*dinv^2
                xwown = xp.tile([128, 128], mybir.dt.float32, tag="xwown")
                nc.sync.dma_start(xwown[:], cc_in[l][t * 128:(t + 1) * 128, :])
                selfh = et.tile([128, 128], mybir.dt.float32, tag="selfh")
                nc.scalar.activation(selfh[:], xwown[:],
                                     mybir.ActivationFunctionType.Copy,
                                     scale=selfw_t[:, t:t + 1])
                h = et.tile([128, 128], mybir.dt.float32, tag="h")
                nc.vector.tensor_tensor(out=h[:], in0=agg_sl, in1=selfh[:],
                                        op=mybir.AluOpType.add)
                nc.vector.tensor_tensor(out=h[:], in0=h[:], in1=b_t[l][:],
                                        op=mybir.AluOpType.add)
                if l in (1, 2):
                    nc.vector.tensor_tensor(out=h[:], in0=h[:], in1=g_sl,
                                            op=mybir.AluOpType.add)
                if l == 3:
                    nc.sync.dma_start(out_dram[t * 128:(t + 1) * 128, :], h[:])
                    return
                nc.scalar.activation(g_sl, h[:],
                                     mybir.ActivationFunctionType.Gelu)
                pgt = ep.tile([128, 128], mybir.dt.float32, space="PSUM",
                              tag="pgt")
                nc.tensor.transpose(out=pgt[:], in_=g_sl, identity=ident_t[:])
                gt_sb = et.tile([128, 128], mybir.dt.float32, tag="gt")
                nc.scalar.activation(gt_sb[:], pgt[:],
                                     mybir.ActivationFunctionType.Copy)
                make_xw(l + 1, gt_sb, t)

            def do_layer(l):
                nc.gpsimd.collective_compute(
                    "AllGather",
                    mybir.AluOpType.bypass,
                    replica_groups=[list(range(NCORES))],
                    ins=[cc_in[l][:].opt()],
                    outs=[cc_out[l][:].opt()],
                )
                table = cc_out[l]
                for (q, t, R, nch, io16, mo) in runs:
                    y = yp.tile([128, max_nch, 128], mybir.dt.float32, tag="y")
                    nwin_rows = min(32768, NT - q * WIN)
                    nc.gpsimd.dma_gather(
                        out_ap=y[:, :nch, :],
                        in_ap=table[q * WIN:q * WIN + nwin_rows, :],
                        idxs_ap=idx_t[:, io16:io16 + R // 16],
                        num_idxs=R,
                        num_idxs_reg=R,
                        elem_size=128,
                        single_packet=False,
                        queue_num=gq[0] % NQUSE,
                    )
                    gq[0] += 1
                    ms = mp.tile([128, max_nch * 128], mybir.dt.float32, tag="ms")
                    nc.sync.dma_start(ms[:, :nch * 128],
                                      m_in[:, mo * 128:(mo + nch) * 128])
                    psum = rp.tile([128, 128], mybir.dt.float32, space="PSUM",
                                   tag="rp")
                    for k in range(nch):
                        nc.tensor.matmul(out=psum[:],
                                         lhsT=ms[:, k * 128:(k + 1) * 128],
                                         rhs=y[:, k, :],
                                         start=(k == 0), stop=(k == nch - 1))
                    agg_sl = agg_t[:, t * 128:(t + 1) * 128]
                    if q == first_q[t]:
                        nc.vector.tensor_copy(agg_sl, psum[:])
                    else:
                        nc.vector.tensor_tensor(out=agg_sl, in0=agg_sl,
                                                in1=psum[:],
                                                op=mybir.AluOpType.add)
                    if q == last_q[t]:
                        epilogue(l, t)

            for l in range(4):
                do_layer(l)

    nc.compile()
    return nc


# --------------------------------------------------------------------------
# public entry point
# --------------------------------------------------------------------------

def _host_inputs(x, edge_index, Ws, bs):
    n_real = x.shape[0]
    per_core, runs, CHX, IDXC = preprocess(x, edge_index, n_real)

    xpad = np.zeros((NT, F), dtype=np.float32)
    xpad[:n_real] = np.asarray(x, dtype=np.float32)

    W3p = np.zeros((128, 128), np.float32)
    W3p[:, :C_OUT] = Ws[3]
    Wl = [np.asarray(Ws[0], np.float32), np.asarray(Ws[1], np.float32),
          np.asarray(Ws[2], np.float32), W3p]
    b3p = np.zeros(128, np.float32)
    b3p[:C_OUT] = bs[3]
    bl = [np.asarray(bs[0], np.float32), np.asarray(bs[1], np.float32),
          np.asarray(bs[2], np.float32), b3p]

    ident = np.eye(128, dtype=np.float32)

    in_maps = []
    for c in range(NCORES):
        d = per_core[c]
        m = {
            "xT": xpad[c * OWN:(c + 1) * OWN].T.copy(),
            "idx16": d["idx16"],
            "M": d["M"],
            "selfw": d["selfw"],
            "ident": ident,
        }
        for l in range(4):
            m[f"W{l}"] = Wl[l]
            m[f"b{l}"] = np.tile(bl[l], (128, 1))
        in_maps.append(m)
    return in_maps, runs, CHX, IDXC


def kernel(x, edge_index, W0, b0, W1, b1, W2, b2, W3, b3):
    x = np.asarray(x)
    in_maps, runs, CHX, IDXC = _host_inputs(
        x, np.asarray(edge_index), [W0, W1, W2, W3], [b0, b1, b2, b3])
    nc = build(runs, CHX, IDXC)
    res = run_bass_kernel_spmd(nc, in_maps, list(range(NCORES)))
    outs = [res.results[c]["out"] for c in range(NCORES)]
    full = np.concatenate(outs, axis=0)[:x.shape[0], :C_OUT]
    return full.astype(np.float32)


# revision 7
# speedup vs baseline: 263.6059x; 263.6059x over previous
"""GCN node classification on 8 Trainium2 NeuronCores (Bass/Tile).

Strategy (dst-sharded graph parallel), v3:
  - Nodes padded to 100352 = 8 * 12544; core c owns dst nodes
    [c*12544, (c+1)*12544)  (98 tiles of 128).
  - Per layer: each core computes xw = g_own @ W on PE; an AllGather makes
    the full [100352, F] feature table resident on every core's HBM.
  - Edges are bucketed by (dst-tile, 25088-row src window = "run"), sorted
    by src; run lengths are the max over the 8 cores (SPMD-uniform
    schedule), rounded to 16; shorter cores pad with idx 0 and zero rows
    in M. Runs are packed into one index stream per window; dma_gather
    calls of up to 2048 idxs pull source rows (int16 idx, relative to the
    window). Host-precomputed selection matrices M[e,d] = coef[e] *
    (d == dst_local[e]) are DMA-streamed from HBM (one [128,128] tile per
    chunk x run segment), and PE accumulates psum += M^T @ Y — the
    weighted segment sum. Chunks that straddle run boundaries issue one
    matmul per overlapped run. Runs flush psum into an SBUF aggregate.
    GPSIMD descriptor generation (~7 ns/idx + ~2-3 us/call) is the
    bottleneck, hence big calls and minimal index padding.
  - Self-loop term: per tile, ACT rescales the core's own xw rows
    (scale = 2*dinv^2 per node) read back from the collective input.
  - Epilogue per tile: + self + bias, + residual, erf-GELU, PE transpose,
    next layer's matmul, DMA into the next collective's input buffer.
"""
import sys

sys.path.insert(0, "/opt/trn_rl_repo")

import numpy as np

import concourse.bass as bass  # noqa: E402
import concourse.tile as tile  # noqa: E402
from concourse import bacc, mybir  # noqa: E402
from concourse.bass_utils import run_bass_kernel_spmd  # noqa: E402

NCORES = 8
F = 128          # feature width (all layers padded to 128)
TILES = 98       # dst tiles per core
OWN = TILES * 128            # 12544 nodes per core
NT = NCORES * OWN            # 100352 padded nodes
NWIN = 4
WIN = 25088                  # src window (int16-addressable, < 32768)
GCALL = 2048                 # idxs per dma_gather call
C_OUT = 40
YBUFS = 4
MBUFS = 4
NQUSE = 4


# --------------------------------------------------------------------------
# host-side schedule
# --------------------------------------------------------------------------

class Sched:
    """Shared (core-independent) schedule.

    runs:  list of dicts {q, t, R, s_lo (stream pos within q), first/last}
    calls: list of dicts {q, lo, n, chunks: [ {slot, segs: [
               {run_idx, e_lo, e_hi, m_idx, start, stop} ] } ]}
    MX:    total number of M tiles
    IDXC:  int16 idx columns (sum over calls of n/16)
    """


def make_schedule(R_tq):
    sched = Sched()
    sched.runs = []
    qlen = [0] * NWIN
    for q in range(NWIN):
        pos = 0
        for t in range(TILES):
            R = int(R_tq[t, q])
            if R == 0:
                continue
            sched.runs.append(dict(q=q, t=t, R=R, s_lo=pos, idx=len(sched.runs)))
            pos += R
        qlen[q] = pos

    first_q, last_q = {}, {}
    for r in sched.runs:
        first_q.setdefault(r["t"], r["q"])
        last_q[r["t"]] = r["q"]
    sched.first_q, sched.last_q = first_q, last_q

    # runs of each q sorted by s_lo already
    runs_by_q = [[r for r in sched.runs if r["q"] == q] for q in range(NWIN)]

    sched.calls = []
    m_idx = 0
    idxc = 0
    for q in range(NWIN):
        rq = runs_by_q[q]
        pos = 0
        ri = 0
        while pos < qlen[q]:
            n = min(GCALL, qlen[q] - pos)
            call = dict(q=q, lo=pos, n=n, idx_off16=idxc, chunks=[])
            idxc += n // 16
            nch = (n + 127) // 128
            for k in range(nch):
                c_lo = pos + k * 128
                c_hi = min(pos + (k + 1) * 128, pos + n)
                segs = []
                # advance ri to first run overlapping c_lo
                while ri < len(rq) and rq[ri]["s_lo"] + rq[ri]["R"] <= c_lo:
                    ri += 1
                rj = ri
                while rj < len(rq) and rq[rj]["s_lo"] < c_hi:
                    r = rq[rj]
                    e_lo = max(r["s_lo"], c_lo) - c_lo
                    e_hi = min(r["s_lo"] + r["R"], c_hi) - c_lo
                    segs.append(dict(
                        run=r, e_lo=e_lo, e_hi=e_hi, m_idx=m_idx,
                        start=(max(r["s_lo"], c_lo) == r["s_lo"]),
                        stop=(min(r["s_lo"] + r["R"], c_hi) == r["s_lo"] + r["R"]),
                    ))
                    m_idx += 1
                    rj += 1
                call["chunks"].append(dict(slot=k, segs=segs))
            sched.calls.append(call)
            pos += n
    sched.MX = m_idx
    sched.IDXC = idxc
    sched.max_nm = max(sum(len(c["segs"]) for c in call["chunks"])
                       for call in sched.calls)
    sched.qlen = qlen
    return sched


def preprocess(x, edge_index, n_real):
    src = np.asarray(edge_index[0], dtype=np.int64)
    dst = np.asarray(edge_index[1], dtype=np.int64)

    deg = np.bincount(dst, minlength=NT).astype(np.float32) + 2.0
    dinv = 1.0 / np.sqrt(deg)
    coef_a = (dinv[src] * dinv[dst]).astype(np.float32)

    core = dst // OWN
    dstl_a = dst - core * OWN
    t_a = dstl_a >> 7
    dloc_a = (dstl_a & 127).astype(np.int64)
    q_a = np.minimum(src // WIN, NWIN - 1)
    idxrel_a = (src - q_a * WIN).astype(np.int64)
    assert idxrel_a.max() < 32768

    counts = np.zeros((NCORES, TILES, NWIN), dtype=np.int64)
    np.add.at(counts, (core, t_a, q_a), 1)
    R_tq = ((counts.max(axis=0) + 15) // 16 * 16).astype(np.int64)

    sched = make_schedule(R_tq)

    order = np.lexsort((idxrel_a, t_a, q_a, core))
    src_s = idxrel_a[order]
    core_s = core[order]
    t_s = t_a[order]
    q_s = q_a[order]
    dloc_s = dloc_a[order]
    coef_s = coef_a[order]

    run_pos = {(r["q"], r["t"]): r for r in sched.runs}
    # stream-global base per q
    qbase = np.cumsum([0] + sched.qlen[:-1])

    per_core = []
    for c in range(NCORES):
        sel = core_s == c
        ci, ct, cq = src_s[sel], t_s[sel], q_s[sel]
        cd, cc = dloc_s[sel], coef_s[sel]
        # flat global stream of idx / dloc / coef (padded)
        SL = int(sum(sched.qlen))
        idx_flat = np.zeros(SL, dtype=np.int16)
        dl_flat = np.zeros(SL, dtype=np.int64)
        cf_flat = np.zeros(SL, dtype=np.float32)
        key = cq * TILES + ct
        bounds = np.flatnonzero(np.r_[True, key[1:] != key[:-1], True])
        for bi in range(len(bounds) - 1):
            lo, hi = bounds[bi], bounds[bi + 1]
            r = run_pos[(int(cq[lo]), int(ct[lo]))]
            n = hi - lo
            assert n <= r["R"]
            g0 = qbase[r["q"]] + r["s_lo"]
            idx_flat[g0:g0 + n] = ci[lo:hi]
            dl_flat[g0:g0 + n] = cd[lo:hi]
            cf_flat[g0:g0 + n] = cc[lo:hi]
        # idx wrapped per call
        idx_w = np.zeros((128, sched.IDXC), dtype=np.int16)
        for call in sched.calls:
            g0 = qbase[call["q"]] + call["lo"]
            blk = idx_flat[g0:g0 + call["n"]].reshape(-1, 16).T
            o = call["idx_off16"]
            idx_w[:, o:o + call["n"] // 16] = np.tile(blk, (8, 1))
        # M tiles per segment
        M = np.zeros((sched.MX, 128, 128), dtype=np.float32)
        for call in sched.calls:
            g0 = qbase[call["q"]] + call["lo"]
            for ch in call["chunks"]:
                c_lo = g0 + ch["slot"] * 128
                for s in ch["segs"]:
                    e = np.arange(s["e_lo"], s["e_hi"])
                    gpos = c_lo + e
                    m = M[s["m_idx"]]
                    m[e, dl_flat[gpos]] = cf_flat[gpos]
        M_w = M.transpose(1, 0, 2).reshape(128, sched.MX * 128)
        own = slice(c * OWN, (c + 1) * OWN)
        selfw = (2.0 * dinv[own] * dinv[own]).astype(np.float32)
        per_core.append({
            "idx16": np.ascontiguousarray(idx_w),
            "M": np.ascontiguousarray(M_w),
            "selfw": selfw.reshape(TILES, 128).T.copy(),
        })

    return per_core, sched


# --------------------------------------------------------------------------
# bass program
# --------------------------------------------------------------------------

def build(sched):
    nc = bacc.Bacc("TRN2", target_bir_lowering=False, debug=False,
                   num_devices=NCORES, num_swdge_queues=NQUSE)

    MX, IDXC = sched.MX, sched.IDXC
    xT_in = nc.dram_tensor("xT", [128, OWN], mybir.dt.float32, kind="ExternalInput")
    idx16_in = nc.dram_tensor("idx16", [128, IDXC], mybir.dt.int16, kind="ExternalInput")
    m_in = nc.dram_tensor("M", [128, MX * 128], mybir.dt.float32, kind="ExternalInput")
    selfw_in = nc.dram_tensor("selfw", [128, TILES], mybir.dt.float32, kind="ExternalInput")
    w_in = [nc.dram_tensor(f"W{l}", [128, 128], mybir.dt.float32, kind="ExternalInput")
            for l in range(4)]
    b_in = [nc.dram_tensor(f"b{l}", [128, 128], mybir.dt.float32, kind="ExternalInput")
            for l in range(4)]
    ident_in = nc.dram_tensor("ident", [128, 128], mybir.dt.float32, kind="ExternalInput")
    out_dram = nc.dram_tensor("out", [OWN, 128], mybir.dt.float32, kind="ExternalOutput")

    max_call_chunks = max((c["n"] + 127) // 128 for c in sched.calls)
    # M tiles per call
    call_m0 = []
    for call in sched.calls:
        first_seg = call["chunks"][0]["segs"][0]["m_idx"]
        nm = sum(len(ch["segs"]) for ch in call["chunks"])
        call_m0.append((first_seg, nm))
    max_nm = sched.max_nm

    with tile.TileContext(nc) as tc:
        with (
            tc.tile_pool(name="persist", bufs=1) as pers,
            tc.tile_pool(name="ybuf", bufs=YBUFS) as yp,
            tc.tile_pool(name="mbuf", bufs=MBUFS) as mp,
            tc.tile_pool(name="ibuf", bufs=4) as ip,
            tc.tile_pool(name="runp", bufs=4, space="PSUM") as rp,
            tc.tile_pool(name="epip", bufs=2, space="PSUM") as ep,
            tc.tile_pool(name="etmp", bufs=4) as et,
            tc.tile_pool(name="xtile", bufs=4) as xp,
            tc.tile_pool(name="dram", bufs=1, space="DRAM") as dp,
        ):
            # ---- persistent SBUF ----
            selfw_t = pers.tile([128, TILES], mybir.dt.float32, tag="selfw")
            nc.sync.dma_start(selfw_t[:], selfw_in[:])
            ident_t = pers.tile([128, 128], mybir.dt.float32, tag="ident")
            nc.sync.dma_start(ident_t[:], ident_in[:])
            w_t, b_t = [], []
            for l in range(4):
                wt = pers.tile([128, 128], mybir.dt.float32, tag=f"w{l}")
                nc.sync.dma_start(wt[:], w_in[l][:])
                w_t.append(wt)
                bt = pers.tile([128, 128], mybir.dt.float32, tag=f"b{l}")
                nc.sync.dma_start(bt[:], b_in[l][:])
                b_t.append(bt)
            agg_t = pers.tile([128, TILES * 128], mybir.dt.float32, tag="agg")
            g_t = pers.tile([128, TILES * 128], mybir.dt.float32, tag="g")

            # zero gather slots once (short-count gathers leave stale tails;
            # M zero rows null them unless stale bits are NaN)
            for _ in range(YBUFS):
                yz = yp.tile([128, max_call_chunks, 128], mybir.dt.float32,
                             tag="y")
                nc.vector.memset(yz[:], 0.0)

            # ---- collective buffers ----
            cc_in = [dp.tile([OWN, 128], mybir.dt.float32, tag=f"ccin{l}",
                             name=f"ccin{l}") for l in range(4)]
            cc_out = [dp.tile([NT, 128], mybir.dt.float32, tag=f"ccout{l}",
                              name=f"ccout{l}", addr_space="Shared")
                      for l in range(4)]

            def make_xw(l, lhsT_tile, t):
                pxw = ep.tile([128, 128], mybir.dt.float32, space="PSUM", tag="pxw")
                nc.tensor.matmul(out=pxw[:], lhsT=lhsT_tile[:], rhs=w_t[l][:],
                                 start=True, stop=True)
                xw_sb = et.tile([128, 128], mybir.dt.float32, tag="xwsb")
                nc.scalar.activation(xw_sb[:], pxw[:],
                                     mybir.ActivationFunctionType.Copy)
                nc.sync.dma_start(cc_in[l][t * 128:(t + 1) * 128, :], xw_sb[:])

            # ---- layer 0 pre-phase: xw0 = x @ W0 ----
            for t in range(TILES):
                xt = xp.tile([128, 128], mybir.dt.float32, tag="xt")
                nc.sync.dma_start(xt[:], xT_in[:, t * 128:(t + 1) * 128])
                make_xw(0, xt, t)

            gq = [0]

            def epilogue(l, t):
                agg_sl = agg_t[:, t * 128:(t + 1) * 128]
                g_sl = g_t[:, t * 128:(t + 1) * 128]
                xwown = xp.tile([128, 128], mybir.dt.float32, tag="xwown")
                nc.sync.dma_start(xwown[:], cc_in[l][t * 128:(t + 1) * 128, :])
                selfh = et.tile([128, 128], mybir.dt.float32, tag="selfh")
                nc.scalar.activation(selfh[:], xwown[:],
                                     mybir.ActivationFunctionType.Copy,
                                     scale=selfw_t[:, t:t + 1])
                h = et.tile([128, 128], mybir.dt.float32, tag="h")
                nc.vector.tensor_tensor(out=h[:], in0=agg_sl, in1=selfh[:],
                                        op=mybir.AluOpType.add)
                nc.vector.tensor_tensor(out=h[:], in0=h[:], in1=b_t[l][:],
                                        op=mybir.AluOpType.add)
                if l in (1, 2):
                    nc.vector.tensor_tensor(out=h[:], in0=h[:], in1=g_sl,
                                            op=mybir.AluOpType.add)
                if l == 3:
                    nc.sync.dma_start(out_dram[t * 128:(t + 1) * 128, :], h[:])
                    return
                nc.scalar.activation(g_sl, h[:],
                                     mybir.ActivationFunctionType.Gelu)
                pgt = ep.tile([128, 128], mybir.dt.float32, space="PSUM",
                              tag="pgt")
                nc.tensor.transpose(out=pgt[:], in_=g_sl, identity=ident_t[:])
                gt_sb = et.tile([128, 128], mybir.dt.float32, tag="gt")
                nc.scalar.activation(gt_sb[:], pgt[:],
                                     mybir.ActivationFunctionType.Copy)
                make_xw(l + 1, gt_sb, t)

            def do_layer(l):
                nc.gpsimd.collective_compute(
                    "AllGather",
                    mybir.AluOpType.bypass,
                    replica_groups=[list(range(NCORES))],
                    ins=[cc_in[l][:].opt()],
                    outs=[cc_out[l][:].opt()],
                )
                table = cc_out[l]
                psum_of_run = {}
                for ci, call in enumerate(sched.calls):
                    q, n = call["q"], call["n"]
                    nch = (n + 127) // 128
                    y = yp.tile([128, max_call_chunks, 128],
                                mybir.dt.float32, tag="y")
                    nwin_rows = min(32768, NT - q * WIN)
                    o16 = call["idx_off16"]
                    idxs = ip.tile([128, max(GCALL // 16, 16)],
                                   mybir.dt.int16, tag="idxs")
                    nc.sync.dma_start(idxs[:, :n // 16],
                                      idx16_in[:, o16:o16 + n // 16])
                    nc.gpsimd.dma_gather(
                        out_ap=y[:, :nch, :],
                        in_ap=table[q * WIN:q * WIN + nwin_rows, :],
                        idxs_ap=idxs[:, :n // 16],
                        num_idxs=n,
                        num_idxs_reg=n,
                        elem_size=128,
                        single_packet=False,
                        queue_num=gq[0] % NQUSE,
                    )
                    gq[0] += 1
                    m0, nm = call_m0[ci]
                    ms = mp.tile([128, max_nm * 128], mybir.dt.float32,
                                 tag="ms")
                    nc.sync.dma_start(ms[:, :nm * 128],
                                      m_in[:, m0 * 128:(m0 + nm) * 128])
                    for ch in call["chunks"]:
                        k = ch["slot"]
                        for s in ch["segs"]:
                            r = s["run"]
                            rid = r["idx"]
                            if s["start"]:
                                psum_of_run[rid] = rp.tile(
                                    [128, 128], mybir.dt.float32,
                                    space="PSUM", tag="rp", name="rpt")
                            psum = psum_of_run[rid]
                            mi = s["m_idx"] - m0
                            nc.tensor.matmul(
                                out=psum[:],
                                lhsT=ms[:, mi * 128:(mi + 1) * 128],
                                rhs=y[:, k, :],
                                start=s["start"], stop=s["stop"])
                            if s["stop"]:
                                t = r["t"]
                                agg_sl = agg_t[:, t * 128:(t + 1) * 128]
                                if q == sched.first_q[t]:
                                    nc.vector.tensor_copy(agg_sl, psum[:])
                                else:
                                    nc.vector.tensor_tensor(
                                        out=agg_sl, in0=agg_sl, in1=psum[:],
                                        op=mybir.AluOpType.add)
                                del psum_of_run[rid]
                                if q == sched.last_q[t]:
                                    epilogue(l, t)

            for l in range(4):
                do_layer(l)

    nc.compile()
    return nc


# --------------------------------------------------------------------------
# public entry point
# --------------------------------------------------------------------------

def _host_inputs(x, edge_index, Ws, bs):
    n_real = x.shape[0]
    per_core, sched = preprocess(x, edge_index, n_real)

    xpad = np.zeros((NT, F), dtype=np.float32)
    xpad[:n_real] = np.asarray(x, dtype=np.float32)

    W3p = np.zeros((128, 128), np.float32)
    W3p[:, :C_OUT] = Ws[3]
    Wl = [np.asarray(Ws[0], np.float32), np.asarray(Ws[1], np.float32),
          np.asarray(Ws[2], np.float32), W3p]
    b3p = np.zeros(128, np.float32)
    b3p[:C_OUT] = bs[3]
    bl = [np.asarray(bs[0], np.float32), np.asarray(bs[1], np.float32),
          np.asarray(bs[2], np.float32), b3p]

    ident = np.eye(128, dtype=np.float32)

    in_maps = []
    for c in range(NCORES):
        d = per_core[c]
        m = {
            "xT": xpad[c * OWN:(c + 1) * OWN].T.copy(),
            "idx16": d["idx16"],
            "M": d["M"],
            "selfw": d["selfw"],
            "ident": ident,
        }
        for l in range(4):
            m[f"W{l}"] = Wl[l]
            m[f"b{l}"] = np.tile(bl[l], (128, 1))
        in_maps.append(m)
    return in_maps, sched


def kernel(x, edge_index, W0, b0, W1, b1, W2, b2, W3, b3):
    x = np.asarray(x)
    in_maps, sched = _host_inputs(
        x, np.asarray(edge_index), [W0, W1, W2, W3], [b0, b1, b2, b3])
    nc = build(sched)
    res = run_bass_kernel_spmd(nc, in_maps, list(range(NCORES)))
    outs = [res.results[c]["out"] for c in range(NCORES)]
    full = np.concatenate(outs, axis=0)[:x.shape[0], :C_OUT]
    return full.astype(np.float32)
